# revision 20
# baseline (speedup 1.0000x reference)
"""Multi-head causal attention with RoPE on 8 Trainium2 NeuronCores.

Sharding: data-parallel over batch (B=2) x tensor-parallel over heads
(16 heads -> 4 groups of 4). Core c handles batch c//4, heads
[(c%4)*4, (c%4)*4+4). Each core computes a partial y = attn_out @ W_o
for its head group; the host sums the 4 partials per batch (the "W_o
all-reduce").

Device kernel (per core, all matmuls bf16, fp32 PSUM accumulation):
  - x^T built on-chip via PE transposes (contraction over E needs E on
    partitions).
  - Q^T/K^T/V^T projections in "T layout" (dims on partitions, seq on
    free): out = W_chunk.T @ x^T_chunk accumulated over 8 E-chunks.
  - RoPE: the within-head pair shuffle is folded into a host-side
    permutation of W_q/W_k columns so the rotation partner sits 16
    partitions away inside the same 32-partition quadrant; on device a
    single DVE stream_shuffle + cos/sin multiply-adds apply the
    rotation. Scores are permutation-invariant since Q and K use the
    same permutation.
  - scores^T[t, q] = K^T_tile.T @ Q^T (only t-blocks <= q-block:
    causal skip), exp on ACT (scale=1/32 folded in), causal mask on
    diagonal blocks, P^T @ [V | 1] accumulated in PSUM -> out^T plus
    softmax denominators in one matmul (ones column appended to V).
  - normalize with reciprocal + gpsimd partition_broadcast, then
    y = out_norm^T.T @ W_o chunks.
"""

import os
import sys
from contextlib import ExitStack

import numpy as np

for _p in ("/opt/trn_rl_repo",):
    if os.path.isdir(_p) and _p not in sys.path:
        sys.path.insert(0, _p)

import ml_dtypes  # noqa: E402

BF16 = ml_dtypes.bfloat16

B, S, E = 2, 2048, 1024
H, DH = 16, 64
NCORES = 8
HPC = H // 4          # 4 heads per core
DC = HPC * DH         # 256 head dims per core
ATTN_SCALE = 1.0 / 32.0  # 1/sqrt(E)
ROPE_BASE = 10000.0
P = 128
NSB = S // P          # 16 sequence blocks
NEC = E // P          # 8 E chunks
MB = DC // P          # 2 partition blocks of head dims

_PROG = None


def _perm64():
    """perm[j] = original head-dim index stored at permuted position j.

    Quadrant q of the permuted layout holds RoPE pairs i in
    [16q, 16q+16): even elements (2i) at slots 0-15, odd (2i+1) at
    slots 16-31. The rotation partner is then always +-16 partitions
    away within one 32-partition quadrant (stream_shuffle range).
    """
    j = np.arange(64)
    qd, r = j // 32, j % 32
    i = 16 * qd + (r % 16)
    return 2 * i + (r >= 16)


def _cos_sin_tiles():
    pl = np.arange(P) % 64
    qd, r = pl // 32, pl % 32
    i = 16 * qd + (r % 16)
    inv = ROPE_BASE ** (-(2.0 * i) / DH)
    ang = np.arange(S)[None, :] * inv[:, None]          # (128, S)
    sgn = np.where(r < 16, -1.0, 1.0)[:, None]
    return ang, sgn


def _build_program(debug=False):
    import concourse.bacc as bacc
    import concourse.tile as tile
    from concourse import masks, mybir

    f32 = mybir.dt.float32
    bf16 = mybir.dt.bfloat16
    AF = mybir.ActivationFunctionType

    nc = bacc.Bacc("TRN2", target_bir_lowering=False, debug=False)
    xb = nc.dram_tensor("xb", [S, E], bf16, kind="ExternalInput").ap()
    wq = nc.dram_tensor("wq", [E, DC], bf16, kind="ExternalInput").ap()
    wk = nc.dram_tensor("wk", [E, DC], bf16, kind="ExternalInput").ap()
    wv = nc.dram_tensor("wv", [E, DC], bf16, kind="ExternalInput").ap()
    wo = nc.dram_tensor("wo", [DC, E], bf16, kind="ExternalInput").ap()
    cosr = nc.dram_tensor("cosr", [P, S], bf16, kind="ExternalInput").ap()
    sinr = nc.dram_tensor("sinr", [P, S], bf16, kind="ExternalInput").ap()
    cmask = nc.dram_tensor("cmask", [P, P], bf16, kind="ExternalInput").ap()
    y = nc.dram_tensor("y", [S, E], f32, kind="ExternalOutput").ap()
    if debug:
        dbg = {
            "dxT": nc.dram_tensor("dxT", [P, NEC, S], mybir.dt.bfloat16,
                                  kind="ExternalOutput").ap(),
            "dqcT": nc.dram_tensor("dqcT", [P, MB, S], mybir.dt.bfloat16,
                                   kind="ExternalOutput").ap(),
            "dqT": nc.dram_tensor("dqT", [P, MB, S], mybir.dt.bfloat16,
                                  kind="ExternalOutput").ap(),
            "dkT": nc.dram_tensor("dkT", [P, MB, S], mybir.dt.bfloat16,
                                  kind="ExternalOutput").ap(),
            "dvn": nc.dram_tensor("dvn", [P, NSB, HPC, 65], mybir.dt.bfloat16,
                                  kind="ExternalOutput").ap(),
            "donrm": nc.dram_tensor("donrm", [P, MB, S], mybir.dt.bfloat16,
                                    kind="ExternalOutput").ap(),
            "dacc": nc.dram_tensor("dacc", [DH, HPC, S], f32,
                                   kind="ExternalOutput").ap(),
            "dden": nc.dram_tensor("dden", [1, HPC, S], f32,
                                   kind="ExternalOutput").ap(),
        }

    with ExitStack() as ctx:
        tc = ctx.enter_context(tile.TileContext(nc))
        consts = ctx.enter_context(tc.tile_pool(name="consts", bufs=1))
        persist = ctx.enter_context(tc.tile_pool(name="persist", bufs=1))

        ident = consts.tile([P, P], bf16, tag="ident")
        masks.make_identity(nc, ident[:])
        cos_t = consts.tile([P, S], bf16, tag="cos")
        nc.sync.dma_start(cos_t[:], cosr)
        sin_t = consts.tile([P, S], bf16, tag="sin")
        nc.sync.dma_start(sin_t[:], sinr)
        msk_t = consts.tile([P, P], bf16, tag="msk")
        nc.sync.dma_start(msk_t[:], cmask)
        wq_t = consts.tile([P, NEC, DC], bf16, tag="wq")
        nc.sync.dma_start(wq_t[:], wq.rearrange("(c p) m -> p c m", p=P))
        wk_t = consts.tile([P, NEC, DC], bf16, tag="wk")
        nc.sync.dma_start(wk_t[:], wk.rearrange("(c p) m -> p c m", p=P))
        wv_t = consts.tile([P, NEC, DC], bf16, tag="wv")
        nc.sync.dma_start(wv_t[:], wv.rearrange("(c p) m -> p c m", p=P))
        wo_t = consts.tile([P, MB, E], bf16, tag="wo")
        nc.sync.dma_start(wo_t[:], wo.rearrange("(c p) n -> p c n", p=P))

        xT = persist.tile([P, NEC, S], bf16, tag="xT")
        qcT = persist.tile([P, MB, S], bf16, tag="qcT")
        kcT = persist.tile([P, MB, S], bf16, tag="kcT")
        vT = persist.tile([P, MB, S], bf16, tag="vT")
        qT = persist.tile([P, MB, S], bf16, tag="qT")
        kT = persist.tile([P, MB, S], bf16, tag="kT")
        vn = persist.tile([P, NSB, HPC, 65], bf16, tag="vn")
        onrm = persist.tile([P, MB, S], bf16, tag="onrm")

        # ---- Phase A: x^T, projections, RoPE, V natural ----
        with ExitStack() as actx:
            xnat = actx.enter_context(tc.tile_pool(name="xnat", bufs=3))
            tp_ps = actx.enter_context(
                tc.tile_pool(name="tp_ps", bufs=3, space="PSUM")
            )
            pr_ps = actx.enter_context(
                tc.tile_pool(name="pr_ps", bufs=2, space="PSUM")
            )
            rtmp = actx.enter_context(tc.tile_pool(name="rtmp", bufs=2))

            # x^T via hardware DMA transpose (xbar): 8 column-chunk
            # transposes straight from DRAM, no PE/DVE involvement.
            for ec in range(NEC):
                nc.sync.dma_start(
                    xT[:, ec, :],
                    xb[:, ec * P:(ec + 1) * P],
                    transpose=True,
                )

            for wt, dst in ((wq_t, qcT), (wk_t, kcT), (wv_t, vT)):
                for mb in range(MB):
                    for half in range(2):
                        ps = pr_ps.tile([P, S // 2], f32, tag="proj")
                        for ec in range(NEC):
                            for qt in range(2):
                                c0 = half * 1024 + qt * 512
                                nc.tensor.matmul(
                                    ps[:, qt * 512:(qt + 1) * 512],
                                    lhsT=wt[:, ec, mb * P:(mb + 1) * P],
                                    rhs=xT[:, ec, c0:c0 + 512],
                                    start=(ec == 0),
                                    stop=(ec == NEC - 1),
                                )
                        if half == 0:
                            nc.vector.tensor_copy(
                                dst[:, mb, half * 1024:(half + 1) * 1024], ps[:]
                            )
                        else:
                            nc.scalar.copy(
                                dst[:, mb, half * 1024:(half + 1) * 1024], ps[:]
                            )

            shuf_mask = list(range(16, 32)) + list(range(16))
            for src, dst in ((qcT, qT), (kcT, kT)):
                for mb in range(MB):
                    sh = rtmp.tile([P, S], bf16, tag="shuf")
                    nc.vector.stream_shuffle(sh[:], src[:, mb, :], shuf_mask)
                    nc.vector.tensor_mul(sh[:], sh[:], sin_t[:])
                    nc.vector.tensor_mul(dst[:, mb, :], src[:, mb, :], cos_t[:])
                    nc.vector.tensor_add(dst[:, mb, :], dst[:, mb, :], sh[:])

            # V natural layout (t on partitions) + ones column per head
            nc.vector.memset(vn[:, :, :, 64:65], 1.0)
            for mb in range(MB):
                for sb_i in range(NSB):
                    ps = tp_ps.tile([P, P], bf16, tag="tp")
                    nc.tensor.transpose(
                        ps[:], vT[:, mb, sb_i * P:(sb_i + 1) * P], ident[:]
                    )
                    nc.vector.tensor_copy(
                        vn[:, sb_i, 2 * mb:2 * mb + 2, 0:64],
                        ps[:].rearrange("p (a b) -> p a b", a=2),
                    )

        # ---- Phase B: attention, two heads interleaved, q in halves ----
        # PSUM budget: 2 acc tiles (65, 1024) = 2 banks each + 2 sc bufs
        # (128, 1024) = 2 banks each -> 8 banks. Interleaving a head pair
        # keeps TensorE dense enough that HAM stays at full clock while
        # ACT runs the exps.
        with ExitStack() as bctx:
            sc_ps = bctx.enter_context(
                tc.tile_pool(name="sc_ps", bufs=2, space="PSUM")
            )
            ac_ps = bctx.enter_context(
                tc.tile_pool(name="ac_ps", bufs=1, space="PSUM")
            )
            ptp = bctx.enter_context(tc.tile_pool(name="ptp", bufs=4))
            dn = bctx.enter_context(tc.tile_pool(name="dn", bufs=2))

            for hp in range(2):
                heads = (2 * hp, 2 * hp + 1)
                for pss in range(2):
                    q0 = pss * 1024
                    accs = {
                        h: ac_ps.tile([65, 1024], f32, tag=f"acc{h % 2}",
                                      name=f"acc_{h}_{pss}")
                        for h in heads
                    }
                    def issue_pv(h, ti, pt, lo, hi):
                        # one PV piece per PSUM bank; bank bk (global)
                        # is complete at ti == 4*bk+3
                        p0 = lo
                        while p0 < hi:
                            bk = p0 // 512
                            p1 = min(hi, (bk + 1) * 512)
                            nc.tensor.matmul(
                                accs[h][:, p0 - q0:p1 - q0],
                                lhsT=vn[:, ti, h, :],
                                rhs=pt[:, p0 - q0:p1 - q0],
                                start=(ti == 0),
                                stop=(ti == 4 * bk + 3),
                            )
                            p0 = p1

                    # software pipeline: PV consumes the PREVIOUS
                    # iteration's exp output, so TensorE never waits on
                    # ScalarE inside an iteration (keeps the PE dense ->
                    # HAM stays at full clock; exp overlaps fully).
                    pending = []
                    for ti in range(8 if pss == 0 else NSB):
                        t0 = ti * P
                        lo = max(t0, q0)
                        hi = q0 + 1024
                        new = []
                        # Both heads' K=64 score matmuls are row-packed
                        # into the PE array via tile_position (rows 0-63
                        # and 64-127) and run CONCURRENTLY. Besides the
                        # 2x, this keeps full-height row activity: the
                        # HAM clock gate never grants 2.4 GHz to a
                        # stream of half-height (K=64) matmuls.
                        scs = {}
                        for h in heads:
                            scs[h] = sc_ps.tile([P, 1024], f32, tag="sc",
                                                name=f"sc_{h}_{ti}")
                        p0 = lo
                        while p0 < hi:
                            p1 = min(hi, (p0 // 512 + 1) * 512)
                            for h in heads:
                                mb, off = h // 2, (h % 2) * DH
                                nc.tensor.matmul(
                                    scs[h][:, p0 - q0:p1 - q0],
                                    lhsT=kT[off:off + DH, mb, t0:t0 + P],
                                    rhs=qT[off:off + DH, mb, p0:p1],
                                    tile_position=((h % 2) * DH, 0),
                                )
                            p0 = p1
                        for h in heads:
                            mb, off = h // 2, (h % 2) * DH
                            sc = scs[h]
                            pt = ptp.tile([P, 1024], bf16, tag="pt")
                            nc.scalar.activation(
                                pt[:, lo - q0:hi - q0],
                                sc[:, lo - q0:hi - q0],
                                AF.Exp,
                                scale=ATTN_SCALE,
                            )
                            if t0 >= q0:
                                nc.vector.tensor_mul(
                                    pt[:, t0 - q0:t0 - q0 + P],
                                    pt[:, t0 - q0:t0 - q0 + P],
                                    msk_t[:],
                                )
                            new.append((h, ti, pt, lo, hi))
                        for args in pending:
                            issue_pv(*args)
                        pending = new
                    for args in pending:
                        issue_pv(*args)
                    for h in heads:
                        mb, off = h // 2, (h % 2) * DH
                        # copy accumulator out of PSUM fast (frees the
                        # banks for the next pass); broadcast the raw
                        # denominators across partitions on the (idle)
                        # GpSimd engine, then a 2-pass approximate
                        # reciprocal (~22 bits) on DVE. The naive
                        # InstReciprocal is ~6 passes (13us per row).
                        # NOTE: partition_broadcast on HW ignores the AP
                        # partition offset (always reads the tile's
                        # partition 0), so the denominator row must be
                        # staged into its own base-0 tile first.
                        acb = dn.tile([DH, 1024], f32, tag=f"acb{h % 2}")
                        nc.vector.tensor_copy(acb[:], accs[h][0:DH, :])
                        den0 = dn.tile([1, 1024], f32, tag=f"den0{h % 2}")
                        nc.scalar.copy(den0[:], accs[h][64:65, :])
                        denb = dn.tile([DH, 1024], f32, tag="denb")
                        nc.gpsimd.partition_broadcast(denb[:], den0[:])
                        rdb = dn.tile([DH, 1024], f32, tag="rdb")
                        scr = dn.tile([DH, 1024], f32, tag="scr")
                        nc.vector.reciprocal_approx_accurate(
                            rdb[:], denb[:], scr[:]
                        )
                        if debug:
                            nc.sync.dma_start(
                                dbg["dacc"][:, h, q0:q0 + 1024], acb[:]
                            )
                            nc.sync.dma_start(
                                dbg["dden"][:, h, q0:q0 + 1024], den0[:]
                            )
                        nc.vector.tensor_mul(
                            onrm[off:off + DH, mb, q0:q0 + 1024],
                            acb[:],
                            rdb[:],
                        )

        if debug:
            nc.sync.dma_start(dbg["dxT"], xT[:])
            nc.sync.dma_start(dbg["dqcT"], qcT[:])
            nc.sync.dma_start(dbg["dqT"], qT[:])
            nc.sync.dma_start(dbg["dkT"], kT[:])
            nc.sync.dma_start(dbg["dvn"], vn[:])
            nc.sync.dma_start(dbg["donrm"], onrm[:])

        # ---- Phase C: output projection ----
        with ExitStack() as cctx:
            y_ps = cctx.enter_context(
                tc.tile_pool(name="y_ps", bufs=2, space="PSUM")
            )
            yo = cctx.enter_context(tc.tile_pool(name="yo", bufs=3))
            for sb_i in range(NSB):
                yp = y_ps.tile([P, E], f32, tag="yp")
                for mb in range(MB):
                    for half in range(2):
                        nc.tensor.matmul(
                            yp[:, half * 512:(half + 1) * 512],
                            lhsT=onrm[:, mb, sb_i * P:(sb_i + 1) * P],
                            rhs=wo_t[:, mb, half * 512:(half + 1) * 512],
                            start=(mb == 0),
                            stop=(mb == MB - 1),
                        )
                ys = yo.tile([P, E], f32, tag="ys")
                if sb_i % 2 == 0:
                    nc.vector.tensor_copy(ys[:], yp[:])
                else:
                    nc.scalar.copy(ys[:], yp[:])
                nc.sync.dma_start(y[sb_i * P:(sb_i + 1) * P, :], ys[:])

    nc.compile()
    return nc


def get_program():
    global _PROG
    if _PROG is None:
        _PROG = _build_program()
    return _PROG


def make_in_maps(x, W_q, W_k, W_v, W_o):
    perm = _perm64()
    idx_local = (np.arange(DC) // 64) * 64 + perm[np.arange(DC) % 64]
    ang, sgn = _cos_sin_tiles()
    cos_np = np.cos(ang).astype(BF16)
    sin_np = (sgn * np.sin(ang)).astype(BF16)
    # scores tile is (t, q): keep t <= q -> upper triangular incl. diagonal
    cmask_np = np.triu(np.ones((P, P))).astype(BF16)
    in_maps = []
    for c in range(NCORES):
        b, hg = c // 4, c % 4
        base = hg * DC
        in_maps.append(
            dict(
                xb=np.ascontiguousarray(x[b].astype(BF16)),
                wq=np.ascontiguousarray(W_q[:, base + idx_local].astype(BF16)),
                wk=np.ascontiguousarray(W_k[:, base + idx_local].astype(BF16)),
                wv=np.ascontiguousarray(W_v[:, base:base + DC].astype(BF16)),
                wo=np.ascontiguousarray(W_o[base:base + DC, :].astype(BF16)),
                cosr=cos_np,
                sinr=sin_np,
                cmask=cmask_np,
            )
        )
    return in_maps


def kernel(x, W_q, W_k, W_v, W_o, _trace=False, _trace_cores=None):
    from concourse.bass_utils import run_bass_kernel_spmd

    x = np.asarray(x, dtype=np.float32)
    W_q = np.asarray(W_q, dtype=np.float32)
    W_k = np.asarray(W_k, dtype=np.float32)
    W_v = np.asarray(W_v, dtype=np.float32)
    W_o = np.asarray(W_o, dtype=np.float32)

    nc = get_program()
    in_maps = make_in_maps(x, W_q, W_k, W_v, W_o)
    res = run_bass_kernel_spmd(
        nc,
        in_maps,
        list(range(NCORES)),
        trace=_trace,
        trace_cores=_trace_cores,
    )
    y = np.zeros((B, S, E), np.float32)
    for c in range(NCORES):
        y[c // 4] += res.results[c]["y"]
    if _trace:
        return y, res
    return y


# revision 24
# speedup vs baseline: 1.2216x; 1.2216x over previous
"""Multi-head causal attention with RoPE on 8 Trainium2 NeuronCores.

Sharding: data-parallel over batch (B=2) x tensor-parallel over heads
(16 heads -> 4 groups of 4). Core c handles batch c//4, heads
[(c%4)*4, (c%4)*4+4). Each core computes a partial y = attn_out @ W_o
for its head group; the host sums the 4 partials per batch (the "W_o
all-reduce").

Device kernel (per core, all matmuls bf16, fp32 PSUM accumulation):
  - x^T built on-chip via PE transposes (contraction over E needs E on
    partitions).
  - Q^T/K^T/V^T projections in "T layout" (dims on partitions, seq on
    free): out = W_chunk.T @ x^T_chunk accumulated over 8 E-chunks.
  - RoPE: the within-head pair shuffle is folded into a host-side
    permutation of W_q/W_k columns so the rotation partner sits 16
    partitions away inside the same 32-partition quadrant; on device a
    single DVE stream_shuffle + cos/sin multiply-adds apply the
    rotation. Scores are permutation-invariant since Q and K use the
    same permutation.
  - scores^T[t, q] = K^T_tile.T @ Q^T (only t-blocks <= q-block:
    causal skip), exp on ACT (scale=1/32 folded in), causal mask on
    diagonal blocks, P^T @ [V | 1] accumulated in PSUM -> out^T plus
    softmax denominators in one matmul (ones column appended to V).
  - normalize with reciprocal + gpsimd partition_broadcast, then
    y = out_norm^T.T @ W_o chunks.
"""

import os
import sys
from contextlib import ExitStack

import numpy as np

for _p in ("/opt/trn_rl_repo",):
    if os.path.isdir(_p) and _p not in sys.path:
        sys.path.insert(0, _p)

import ml_dtypes  # noqa: E402

BF16 = ml_dtypes.bfloat16

B, S, E = 2, 2048, 1024
H, DH = 16, 64
NCORES = 8
HPC = H // 4          # 4 heads per core
DC = HPC * DH         # 256 head dims per core
ATTN_SCALE = 1.0 / 32.0  # 1/sqrt(E)
ROPE_BASE = 10000.0
P = 128
NSB = S // P          # 16 sequence blocks
NEC = E // P          # 8 E chunks
MB = DC // P          # 2 partition blocks of head dims

_PROG = None


def _perm64():
    """perm[j] = original head-dim index stored at permuted position j.

    Quadrant q of the permuted layout holds RoPE pairs i in
    [16q, 16q+16): even elements (2i) at slots 0-15, odd (2i+1) at
    slots 16-31. The rotation partner is then always +-16 partitions
    away within one 32-partition quadrant (stream_shuffle range).
    """
    j = np.arange(64)
    qd, r = j // 32, j % 32
    i = 16 * qd + (r % 16)
    return 2 * i + (r >= 16)


def _cos_sin_tiles():
    pl = np.arange(P) % 64
    qd, r = pl // 32, pl % 32
    i = 16 * qd + (r % 16)
    inv = ROPE_BASE ** (-(2.0 * i) / DH)
    ang = np.arange(S)[None, :] * inv[:, None]          # (128, S)
    sgn = np.where(r < 16, -1.0, 1.0)[:, None]
    return ang, sgn


def _build_program(debug=False):
    import concourse.bacc as bacc
    import concourse.tile as tile
    from concourse import masks, mybir

    f32 = mybir.dt.float32
    bf16 = mybir.dt.bfloat16
    AF = mybir.ActivationFunctionType

    nc = bacc.Bacc("TRN2", target_bir_lowering=False, debug=False)
    xb = nc.dram_tensor("xb", [S, E], bf16, kind="ExternalInput").ap()
    wq = nc.dram_tensor("wq", [E, DC], bf16, kind="ExternalInput").ap()
    wk = nc.dram_tensor("wk", [E, DC], bf16, kind="ExternalInput").ap()
    wv = nc.dram_tensor("wv", [E, DC], bf16, kind="ExternalInput").ap()
    wo = nc.dram_tensor("wo", [DC, E], bf16, kind="ExternalInput").ap()
    cosr = nc.dram_tensor("cosr", [P, S], bf16, kind="ExternalInput").ap()
    sinr = nc.dram_tensor("sinr", [P, S], bf16, kind="ExternalInput").ap()
    cmask = nc.dram_tensor("cmask", [P, P], bf16, kind="ExternalInput").ap()
    y = nc.dram_tensor("y", [S, E], f32, kind="ExternalOutput").ap()
    if debug:
        dbg = {
            "dxT": nc.dram_tensor("dxT", [P, NEC, S], mybir.dt.bfloat16,
                                  kind="ExternalOutput").ap(),
            "dqcT": nc.dram_tensor("dqcT", [P, MB, S], mybir.dt.bfloat16,
                                   kind="ExternalOutput").ap(),
            "dqT": nc.dram_tensor("dqT", [P, MB, S], mybir.dt.bfloat16,
                                  kind="ExternalOutput").ap(),
            "dkT": nc.dram_tensor("dkT", [P, MB, S], mybir.dt.bfloat16,
                                  kind="ExternalOutput").ap(),
            "dvn": nc.dram_tensor("dvn", [P, NSB, HPC, 65], mybir.dt.bfloat16,
                                  kind="ExternalOutput").ap(),
            "donrm": nc.dram_tensor("donrm", [P, MB, S], mybir.dt.bfloat16,
                                    kind="ExternalOutput").ap(),
            "dacc": nc.dram_tensor("dacc", [DH, HPC, S], f32,
                                   kind="ExternalOutput").ap(),
            "dden": nc.dram_tensor("dden", [1, HPC, S], f32,
                                   kind="ExternalOutput").ap(),
        }

    with ExitStack() as ctx:
        tc = ctx.enter_context(tile.TileContext(nc))
        consts = ctx.enter_context(tc.tile_pool(name="consts", bufs=1))
        persist = ctx.enter_context(tc.tile_pool(name="persist", bufs=1))

        ident = consts.tile([P, P], bf16, tag="ident")
        masks.make_identity(nc, ident[:])
        cos_t = consts.tile([P, S], bf16, tag="cos")
        nc.sync.dma_start(cos_t[:], cosr)
        sin_t = consts.tile([P, S], bf16, tag="sin")
        nc.sync.dma_start(sin_t[:], sinr)
        msk_t = consts.tile([P, P], bf16, tag="msk")
        nc.sync.dma_start(msk_t[:], cmask)
        wq_t = consts.tile([P, NEC, DC], bf16, tag="wq")
        nc.sync.dma_start(wq_t[:], wq.rearrange("(c p) m -> p c m", p=P))
        wk_t = consts.tile([P, NEC, DC], bf16, tag="wk")
        nc.sync.dma_start(wk_t[:], wk.rearrange("(c p) m -> p c m", p=P))
        wv_t = consts.tile([P, NEC, DC], bf16, tag="wv")
        nc.sync.dma_start(wv_t[:], wv.rearrange("(c p) m -> p c m", p=P))
        wo_t = consts.tile([P, MB, E], bf16, tag="wo")
        nc.sync.dma_start(wo_t[:], wo.rearrange("(c p) n -> p c n", p=P))

        xT = persist.tile([P, NEC, S], bf16, tag="xT")
        qcT = persist.tile([P, MB, S], bf16, tag="qcT")
        kcT = persist.tile([P, MB, S], bf16, tag="kcT")
        vT = persist.tile([P, MB, S], bf16, tag="vT")
        # qz holds RoPE'd Q^T zero-padded per head parity: slice
        # [:, mb, par, :] has head (2*mb+par)'s 64 rows live and the
        # other 64 rows zero. Scores then use the FULL 128-row K^T
        # block as lhsT (K=128): the HAM clock gate never grants full
        # clock to half-height (K=64) matmul streams, and the zero
        # rows contribute exactly 0.
        qz = persist.tile([P, MB, 2, S], bf16, tag="qz")
        kT = persist.tile([P, MB, S], bf16, tag="kT")
        vn = persist.tile([P, NSB, HPC, 65], bf16, tag="vn")
        onrm = persist.tile([P, MB, S], bf16, tag="onrm")

        # ---- Phase A: x^T, projections, RoPE, V natural ----
        with ExitStack() as actx:
            xnat = actx.enter_context(tc.tile_pool(name="xnat", bufs=3))
            tp_ps = actx.enter_context(
                tc.tile_pool(name="tp_ps", bufs=3, space="PSUM")
            )
            pr_ps = actx.enter_context(
                tc.tile_pool(name="pr_ps", bufs=2, space="PSUM")
            )
            rtmp = actx.enter_context(tc.tile_pool(name="rtmp", bufs=2))

            # x^T via hardware DMA transpose (xbar): 8 column-chunk
            # transposes straight from DRAM, no PE/DVE involvement.
            for ec in range(NEC):
                nc.sync.dma_start(
                    xT[:, ec, :],
                    xb[:, ec * P:(ec + 1) * P],
                    transpose=True,
                )

            for wt, dst in ((wq_t, qcT), (wk_t, kcT), (wv_t, vT)):
                for mb in range(MB):
                    for half in range(2):
                        ps = pr_ps.tile([P, S // 2], f32, tag="proj")
                        for ec in range(NEC):
                            for qt in range(2):
                                c0 = half * 1024 + qt * 512
                                nc.tensor.matmul(
                                    ps[:, qt * 512:(qt + 1) * 512],
                                    lhsT=wt[:, ec, mb * P:(mb + 1) * P],
                                    rhs=xT[:, ec, c0:c0 + 512],
                                    start=(ec == 0),
                                    stop=(ec == NEC - 1),
                                )
                        if half == 0:
                            nc.vector.tensor_copy(
                                dst[:, mb, half * 1024:(half + 1) * 1024], ps[:]
                            )
                        else:
                            nc.scalar.copy(
                                dst[:, mb, half * 1024:(half + 1) * 1024], ps[:]
                            )

            shuf_mask = list(range(16, 32)) + list(range(16))
            nc.gpsimd.memset(qz[0:DH, :, 1, :], 0.0)
            nc.gpsimd.memset(qz[DH:P, :, 0, :], 0.0)
            for mb in range(MB):
                sh = rtmp.tile([P, S], bf16, tag="shuf")
                nc.vector.stream_shuffle(sh[:], qcT[:, mb, :], shuf_mask)
                nc.vector.tensor_mul(sh[:], sh[:], sin_t[:])
                for par in range(2):
                    o0 = par * DH
                    nc.vector.tensor_mul(
                        qz[o0:o0 + DH, mb, par, :],
                        qcT[o0:o0 + DH, mb, :],
                        cos_t[o0:o0 + DH, :],
                    )
                    nc.vector.tensor_add(
                        qz[o0:o0 + DH, mb, par, :],
                        qz[o0:o0 + DH, mb, par, :],
                        sh[o0:o0 + DH, :],
                    )
            for mb in range(MB):
                sh = rtmp.tile([P, S], bf16, tag="shuf")
                nc.vector.stream_shuffle(sh[:], kcT[:, mb, :], shuf_mask)
                nc.vector.tensor_mul(sh[:], sh[:], sin_t[:])
                nc.vector.tensor_mul(kT[:, mb, :], kcT[:, mb, :], cos_t[:])
                nc.vector.tensor_add(kT[:, mb, :], kT[:, mb, :], sh[:])

            # V natural layout (t on partitions) + ones column per head
            nc.vector.memset(vn[:, :, :, 64:65], 1.0)
            for mb in range(MB):
                for sb_i in range(NSB):
                    ps = tp_ps.tile([P, P], bf16, tag="tp")
                    nc.tensor.transpose(
                        ps[:], vT[:, mb, sb_i * P:(sb_i + 1) * P], ident[:]
                    )
                    nc.vector.tensor_copy(
                        vn[:, sb_i, 2 * mb:2 * mb + 2, 0:64],
                        ps[:].rearrange("p (a b) -> p a b", a=2),
                    )

        # ---- Phase B: attention, two heads interleaved, q in halves ----
        # PSUM budget: 2 acc tiles (65, 1024) = 2 banks each + 2 sc bufs
        # (128, 1024) = 2 banks each -> 8 banks. Interleaving a head pair
        # keeps TensorE dense enough that HAM stays at full clock while
        # ACT runs the exps.
        with ExitStack() as bctx:
            sc_ps = bctx.enter_context(
                tc.tile_pool(name="sc_ps", bufs=2, space="PSUM")
            )
            ac_ps = bctx.enter_context(
                tc.tile_pool(name="ac_ps", bufs=1, space="PSUM")
            )
            ptp = bctx.enter_context(tc.tile_pool(name="ptp", bufs=4))
            dn = bctx.enter_context(tc.tile_pool(name="dn", bufs=2))

            for hp in range(2):
                heads = (2 * hp, 2 * hp + 1)
                for pss in range(2):
                    q0 = pss * 1024
                    accs = {
                        h: ac_ps.tile([65, 1024], f32, tag=f"acc{h % 2}",
                                      name=f"acc_{h}_{pss}")
                        for h in heads
                    }
                    def issue_pv(h, ti, pt, lo, hi):
                        # one PV piece per PSUM bank; bank bk (global)
                        # is complete at ti == 4*bk+3
                        p0 = lo
                        while p0 < hi:
                            bk = p0 // 512
                            p1 = min(hi, (bk + 1) * 512)
                            nc.tensor.matmul(
                                accs[h][:, p0 - q0:p1 - q0],
                                lhsT=vn[:, ti, h, :],
                                rhs=pt[:, p0 - q0:p1 - q0],
                                start=(ti == 0),
                                stop=(ti == 4 * bk + 3),
                            )
                            p0 = p1

                    # software pipeline: PV consumes the PREVIOUS
                    # iteration's exp output, so TensorE never waits on
                    # ScalarE inside an iteration (keeps the PE dense ->
                    # HAM stays at full clock; exp overlaps fully).
                    pending = []
                    for ti in range(8 if pss == 0 else NSB):
                        t0 = ti * P
                        lo = max(t0, q0)
                        hi = q0 + 1024
                        new = []
                        scs = {}
                        for h in heads:
                            scs[h] = sc_ps.tile([P, 1024], f32, tag="sc",
                                                name=f"sc_{h}_{ti}")
                        p0 = lo
                        while p0 < hi:
                            p1 = min(hi, (p0 // 512 + 1) * 512)
                            for h in heads:
                                mb = h // 2
                                nc.tensor.matmul(
                                    scs[h][:, p0 - q0:p1 - q0],
                                    lhsT=kT[:, mb, t0:t0 + P],
                                    rhs=qz[:, mb, h % 2, p0:p1],
                                )
                            p0 = p1
                        for h in heads:
                            mb, off = h // 2, (h % 2) * DH
                            sc = scs[h]
                            pt = ptp.tile([P, 1024], bf16, tag="pt")
                            nc.scalar.activation(
                                pt[:, lo - q0:hi - q0],
                                sc[:, lo - q0:hi - q0],
                                AF.Exp,
                                scale=ATTN_SCALE,
                            )
                            if t0 >= q0:
                                nc.vector.tensor_mul(
                                    pt[:, t0 - q0:t0 - q0 + P],
                                    pt[:, t0 - q0:t0 - q0 + P],
                                    msk_t[:],
                                )
                            new.append((h, ti, pt, lo, hi))
                        for args in pending:
                            issue_pv(*args)
                        pending = new
                    for args in pending:
                        issue_pv(*args)
                    for h in heads:
                        mb, off = h // 2, (h % 2) * DH
                        # copy accumulator out of PSUM fast (frees the
                        # banks for the next pass); broadcast the raw
                        # denominators across partitions on the (idle)
                        # GpSimd engine, then a 2-pass approximate
                        # reciprocal (~22 bits) on DVE. The naive
                        # InstReciprocal is ~6 passes (13us per row).
                        # NOTE: partition_broadcast on HW ignores the AP
                        # partition offset (always reads the tile's
                        # partition 0), so the denominator row must be
                        # staged into its own base-0 tile first.
                        acb = dn.tile([DH, 1024], f32, tag=f"acb{h % 2}")
                        nc.vector.tensor_copy(acb[:], accs[h][0:DH, :])
                        den0 = dn.tile([1, 1024], f32, tag=f"den0{h % 2}")
                        nc.scalar.copy(den0[:], accs[h][64:65, :])
                        denb = dn.tile([DH, 1024], f32, tag="denb")
                        nc.gpsimd.partition_broadcast(denb[:], den0[:])
                        rdb = dn.tile([DH, 1024], f32, tag="rdb")
                        scr = dn.tile([DH, 1024], f32, tag="scr")
                        nc.vector.reciprocal_approx_accurate(
                            rdb[:], denb[:], scr[:]
                        )
                        if debug:
                            nc.sync.dma_start(
                                dbg["dacc"][:, h, q0:q0 + 1024], acb[:]
                            )
                            nc.sync.dma_start(
                                dbg["dden"][:, h, q0:q0 + 1024], den0[:]
                            )
                        nc.vector.tensor_mul(
                            onrm[off:off + DH, mb, q0:q0 + 1024],
                            acb[:],
                            rdb[:],
                        )

        if debug:
            nc.sync.dma_start(dbg["dxT"], xT[:])
            nc.sync.dma_start(dbg["dqcT"], qcT[:])
            for _mb in range(MB):
                for _par in range(2):
                    _o0 = _par * DH
                    nc.sync.dma_start(
                        dbg["dqT"][_o0:_o0 + DH, _mb, :],
                        qz[_o0:_o0 + DH, _mb, _par, :],
                    )
            nc.sync.dma_start(dbg["dkT"], kT[:])
            nc.sync.dma_start(dbg["dvn"], vn[:])
            nc.sync.dma_start(dbg["donrm"], onrm[:])

        # ---- Phase C: output projection ----
        with ExitStack() as cctx:
            y_ps = cctx.enter_context(
                tc.tile_pool(name="y_ps", bufs=2, space="PSUM")
            )
            yo = cctx.enter_context(tc.tile_pool(name="yo", bufs=3))
            for sb_i in range(NSB):
                yp = y_ps.tile([P, E], f32, tag="yp")
                for mb in range(MB):
                    for half in range(2):
                        nc.tensor.matmul(
                            yp[:, half * 512:(half + 1) * 512],
                            lhsT=onrm[:, mb, sb_i * P:(sb_i + 1) * P],
                            rhs=wo_t[:, mb, half * 512:(half + 1) * 512],
                            start=(mb == 0),
                            stop=(mb == MB - 1),
                        )
                ys = yo.tile([P, E], f32, tag="ys")
                if sb_i % 2 == 0:
                    nc.vector.tensor_copy(ys[:], yp[:])
                else:
                    nc.scalar.copy(ys[:], yp[:])
                nc.sync.dma_start(y[sb_i * P:(sb_i + 1) * P, :], ys[:])

    nc.compile()
    return nc


def get_program():
    global _PROG
    if _PROG is None:
        _PROG = _build_program()
    return _PROG


def make_in_maps(x, W_q, W_k, W_v, W_o):
    perm = _perm64()
    idx_local = (np.arange(DC) // 64) * 64 + perm[np.arange(DC) % 64]
    ang, sgn = _cos_sin_tiles()
    cos_np = np.cos(ang).astype(BF16)
    sin_np = (sgn * np.sin(ang)).astype(BF16)
    # scores tile is (t, q): keep t <= q -> upper triangular incl. diagonal
    cmask_np = np.triu(np.ones((P, P))).astype(BF16)
    in_maps = []
    for c in range(NCORES):
        b, hg = c // 4, c % 4
        base = hg * DC
        in_maps.append(
            dict(
                xb=np.ascontiguousarray(x[b].astype(BF16)),
                wq=np.ascontiguousarray(W_q[:, base + idx_local].astype(BF16)),
                wk=np.ascontiguousarray(W_k[:, base + idx_local].astype(BF16)),
                wv=np.ascontiguousarray(W_v[:, base:base + DC].astype(BF16)),
                wo=np.ascontiguousarray(W_o[base:base + DC, :].astype(BF16)),
                cosr=cos_np,
                sinr=sin_np,
                cmask=cmask_np,
            )
        )
    return in_maps


def kernel(x, W_q, W_k, W_v, W_o, _trace=False, _trace_cores=None):
    from concourse.bass_utils import run_bass_kernel_spmd

    x = np.asarray(x, dtype=np.float32)
    W_q = np.asarray(W_q, dtype=np.float32)
    W_k = np.asarray(W_k, dtype=np.float32)
    W_v = np.asarray(W_v, dtype=np.float32)
    W_o = np.asarray(W_o, dtype=np.float32)

    nc = get_program()
    in_maps = make_in_maps(x, W_q, W_k, W_v, W_o)
    res = run_bass_kernel_spmd(
        nc,
        in_maps,
        list(range(NCORES)),
        trace=_trace,
        trace_cores=_trace_cores,
    )
    y = np.zeros((B, S, E), np.float32)
    for c in range(NCORES):
        y[c // 4] += res.results[c]["y"]
    if _trace:
        return y, res
    return y
